# revision 15
# baseline (speedup 1.0000x reference)
"""Trainium2 Bass kernel for a 2-layer Mamba stack (BasicLayer).

Per layer: LayerNorm -> in_proj (1024->4096) -> causal depthwise conv(k=4)
+ SiLU -> x_proj (2048->96) -> dt_proj + softplus -> selective scan over
L=2048 -> gate with SiLU(z) -> out_proj (2048->1024).

Sharding: tensor-parallel over d_inner (2048 / 8 cores = 256 channels per
core).  The selective scan is independent per channel, so each core scans
its own channels.  Cross-core sums (x_proj contraction and out_proj
contraction over d_inner) are AllReduced on-chip.  Weights are pre-sliced
and pre-transposed on the host (pure data movement); all math runs on
device.
"""

import numpy as np

try:
    import concourse.bass as bass
except ImportError:  # pragma: no cover - fallback for odd sys.path setups
    import sys

    sys.path.insert(0, "/opt/trn_rl_repo")
    import concourse.bass as bass

import concourse.bacc as bacc
import concourse.mybir as mybir
import concourse.tile as tile
from concourse.bass_utils import run_bass_kernel_spmd

F32 = mybir.dt.float32
AF = mybir.ActivationFunctionType
ALU = mybir.AluOpType

# Problem shapes (hardcoded per the contract)
B, L = 2, 2048
DM, DI, DS, DTR, DCONV, DEPTH = 1024, 2048, 16, 64, 4, 2
EPS = 1e-5
NCORES = 8
DL = DI // NCORES          # 256 channels per core
NDT = DL // 128            # 2 channel tiles per core
T = B * L                  # 4096 tokens
NTT = T // 128             # 32 token tiles
NCH = T // 512             # 8 chunks of 512 tokens


def build_nc(apply_norm_w: bool, apply_norm_b: bool, fake_cc: bool = False):
    nc = bacc.Bacc(
        "TRN2",
        target_bir_lowering=False,
        debug=False,
        enable_asserts=False,
        num_devices=NCORES,
    )

    # ---- I/O declarations (per-core data supplied via in_maps) ----
    x_dram = nc.dram_tensor("x_tm", [T, DM], F32, kind="ExternalInput")
    w_inT = nc.dram_tensor("w_inT", [DEPTH, DM, 4 * 128], F32, kind="ExternalInput")
    w_outT = nc.dram_tensor("w_outT", [DEPTH, DL, DM], F32, kind="ExternalInput")
    w_xpT = nc.dram_tensor("w_xpT", [DEPTH, DL, 96], F32, kind="ExternalInput")
    w_dtT = nc.dram_tensor("w_dtT", [DEPTH, DTR, DL], F32, kind="ExternalInput")
    conv_w = nc.dram_tensor("conv_w_c", [DEPTH, DL, DCONV], F32, kind="ExternalInput")
    conv_b = nc.dram_tensor("conv_b_c", [DEPTH, DL, 1], F32, kind="ExternalInput")
    dt_b = nc.dram_tensor("dt_b_c", [DEPTH, DL, 1], F32, kind="ExternalInput")
    a_log = nc.dram_tensor("a_log_c", [DEPTH, DL, DS], F32, kind="ExternalInput")
    d_p = nc.dram_tensor("d_c", [DEPTH, DL, 1], F32, kind="ExternalInput")
    ident = nc.dram_tensor("ident", [128, 128], F32, kind="ExternalInput")
    ones1 = nc.dram_tensor("ones1", [1, 128], F32, kind="ExternalInput")
    if apply_norm_w:
        nwb = nc.dram_tensor("norm_w_bc", [DEPTH, 128, DM], F32, kind="ExternalInput")
    if apply_norm_b:
        nbb = nc.dram_tensor("norm_b_bc", [DEPTH, 128, DM], F32, kind="ExternalInput")
    out_dram = nc.dram_tensor("out_tm", [T, DM], F32, kind="ExternalOutput")

    groups = [list(range(NCORES))]

    with tile.TileContext(nc, num_cores=NCORES) as tc:
        with (
            tc.tile_pool(name="wp", bufs=1) as wp,
            tc.tile_pool(name="lnp", bufs=1) as lnp,
            tc.tile_pool(name="sp", bufs=1) as sp,
            tc.tile_pool(name="dp", bufs=1) as dp,
            tc.tile_pool(name="dram", bufs=2, space="DRAM") as dram,
        ):
            ident_sb = wp.tile([128, 128], F32, tag="ident")
            nc.sync.dma_start(ident_sb[:], ident[:, :])
            ones_sb = wp.tile([1, 128], F32, tag="ones")
            nc.sync.dma_start(ones_sb[:], ones1[:, :])
            eps_sb = wp.tile([128, 1], F32, tag="eps")
            nc.vector.memset(eps_sb[:], EPS)

            h_src = x_dram.ap()
            for l in range(DEPTH):
                # ---- per-layer weights ----
                winT = []
                for kt in range(8):
                    w = wp.tile([128, 512], F32, tag=f"winT{kt}")
                    nc.sync.dma_start(w[:], w_inT[l, kt * 128:(kt + 1) * 128, :])
                    winT.append(w)
                woutT = []
                for j in range(NDT):
                    w = wp.tile([128, DM], F32, tag=f"woutT{j}")
                    nc.sync.dma_start(w[:], w_outT[l, j * 128:(j + 1) * 128, :])
                    woutT.append(w)
                wxpT = []
                for j in range(NDT):
                    w = wp.tile([128, 96], F32, tag=f"wxpT{j}")
                    nc.sync.dma_start(w[:], w_xpT[l, j * 128:(j + 1) * 128, :])
                    wxpT.append(w)
                wdtT = wp.tile([DTR, DL], F32, tag="wdtT")
                nc.sync.dma_start(wdtT[:], w_dtT[l, :, :])
                convw, convb, dtb, Dp, Asb = [], [], [], [], []
                for j in range(NDT):
                    cw = wp.tile([128, DCONV], F32, tag=f"convw{j}")
                    nc.sync.dma_start(cw[:], conv_w[l, j * 128:(j + 1) * 128, :])
                    convw.append(cw)
                    cb = wp.tile([128, 1], F32, tag=f"convb{j}")
                    nc.sync.dma_start(cb[:], conv_b[l, j * 128:(j + 1) * 128, :])
                    convb.append(cb)
                    db = wp.tile([128, 1], F32, tag=f"dtb{j}")
                    nc.sync.dma_start(db[:], dt_b[l, j * 128:(j + 1) * 128, :])
                    dtb.append(db)
                    dd = wp.tile([128, 1], F32, tag=f"dd{j}")
                    nc.sync.dma_start(dd[:], d_p[l, j * 128:(j + 1) * 128, :])
                    Dp.append(dd)
                    at = wp.tile([128, DS], F32, tag=f"alog{j}")
                    nc.sync.dma_start(at[:], a_log[l, j * 128:(j + 1) * 128, :])
                    ae = wp.tile([128, DS], F32, tag=f"aexp{j}")
                    nc.scalar.activation(ae[:], at[:], AF.Exp)
                    an = wp.tile([128, DS], F32, tag=f"aneg{j}")
                    nc.vector.tensor_scalar_mul(an[:], ae[:], -1.0)
                    Asb.append(an)
                if apply_norm_w:
                    nw_sb = wp.tile([128, DM], F32, tag="nwsb")
                    nc.sync.dma_start(nw_sb[:], nwb[l, :, :])
                if apply_norm_b:
                    nb_sb = wp.tile([128, DM], F32, tag="nbsb")
                    nc.sync.dma_start(nb_sb[:], nbb[l, :, :])

                # ---- DRAM staging for this layer ----
                u_st = dram.tile([DL, T], F32, tag="ust")
                z_st = dram.tile([DL, T], F32, tag="zst")
                out_part = dram.tile([T, DM], F32, tag="opart")
                xdbl_in = dram.tile([96, T], F32, tag="xdbli")
                xdbl_out = dram.tile([96, T], F32, tag="xdblo", addr_space="Shared")
                hred = dram.tile([T, DM], F32, tag="hred", addr_space="Shared")

                # ================= phase A: LN + transpose + in_proj + conv =================
                x_dbl = sp.tile([96, T], F32, tag="xdbl")
                prev_uext = [None, None]
                with tc.tile_pool(name=f"psA{l}", bufs=2, space="PSUM") as psA:
                    for ci in range(NCH):
                        tok0 = ci * 512
                        hn_pack = lnp.tile([128, 4096], F32, tag="hnpack")
                        for tti in range(4):
                            tt = ci * 4 + tti
                            xa = lnp.tile([128, DM], F32, tag="xa")
                            nc.sync.dma_start(xa[:], h_src[tt * 128:(tt + 1) * 128, :])
                            st6 = lnp.tile([128, 12], F32, tag="st6", bufs=2)
                            nc.vector.bn_stats(st6[:, 0:6], xa[:, 0:512])
                            nc.vector.bn_stats(st6[:, 6:12], xa[:, 512:1024])
                            mv = lnp.tile([128, 2], F32, tag="mv", bufs=2)
                            nc.vector.bn_aggr(mv[:], st6[:].rearrange("p (g s) -> p g s", g=2))
                            std = lnp.tile([128, 1], F32, tag="std", bufs=2)
                            nc.scalar.activation(std[:], mv[:, 1:2], AF.Sqrt, bias=eps_sb[:])
                            rstd = lnp.tile([128, 1], F32, tag="rstd", bufs=2)
                            nc.vector.reciprocal(rstd[:], std[:])
                            nbias = lnp.tile([128, 1], F32, tag="nbias", bufs=2)
                            nc.vector.scalar_tensor_tensor(
                                nbias[:], mv[:, 0:1], -1.0, rstd[:], ALU.mult, ALU.mult
                            )
                            hcol = hn_pack[:, tti * DM:(tti + 1) * DM]
                            if apply_norm_w or apply_norm_b:
                                hn0 = lnp.tile([128, DM], F32, tag="hn0", bufs=2)
                                nc.scalar.activation(
                                    hn0[:], xa[:], AF.Identity, bias=nbias[:], scale=rstd[:]
                                )
                                if apply_norm_w and apply_norm_b:
                                    hn1 = lnp.tile([128, DM], F32, tag="hn1", bufs=2)
                                    nc.vector.tensor_mul(hn1[:], hn0[:], nw_sb[:])
                                    nc.vector.tensor_add(hcol, hn1[:], nb_sb[:])
                                elif apply_norm_w:
                                    nc.vector.tensor_mul(hcol, hn0[:], nw_sb[:])
                                else:
                                    nc.vector.tensor_add(hcol, hn0[:], nb_sb[:])
                            else:
                                nc.scalar.activation(
                                    hcol, xa[:], AF.Identity, bias=nbias[:], scale=rstd[:]
                                )
                        hnT = []
                        for kt in range(8):
                            pt = psA.tile([128, 512], F32, tag="pt")
                            for tti in range(4):
                                nc.tensor.transpose(
                                    pt[:, tti * 128:(tti + 1) * 128],
                                    hn_pack[:, tti * DM + kt * 128: tti * DM + (kt + 1) * 128],
                                    ident_sb[:],
                                )
                            ht = lnp.tile([128, 512], F32, tag=f"hnT{kt}")
                            nc.any.tensor_copy(ht[:], pt[:])
                            hnT.append(ht)
                        for mt in range(4):
                            pm = psA.tile([128, 512], F32, tag="pm")
                            for kt in range(8):
                                nc.tensor.matmul(
                                    pm[:],
                                    winT[kt][:, mt * 128:(mt + 1) * 128],
                                    hnT[kt][:],
                                    start=(kt == 0),
                                    stop=(kt == 7),
                                )
                            if mt < NDT:
                                ue = sp.tile([128, 515], F32, tag=f"uext{mt}", bufs=2)
                                if ci % 4 == 0:
                                    nc.vector.memset(ue[:, 0:3], 0.0)
                                else:
                                    nc.vector.tensor_copy(
                                        ue[:, 0:3], prev_uext[mt][:, 512:515]
                                    )
                                nc.any.tensor_copy(ue[:, 3:515], pm[:])
                                prev_uext[mt] = ue
                            else:
                                zc = sp.tile([128, 512], F32, tag="zc")
                                nc.scalar.activation(zc[:], pm[:], AF.Silu)
                                nc.sync.dma_start(
                                    z_st[(mt - NDT) * 128:(mt - NDT + 1) * 128, tok0:tok0 + 512],
                                    zc[:],
                                )
                        px = psA.tile([96, 512], F32, tag="px")
                        for j in range(NDT):
                            ue = prev_uext[j]
                            c0 = sp.tile([128, 512], F32, tag="cv0")
                            nc.vector.tensor_scalar(
                                c0[:], ue[:, 0:512], convw[j][:, 0:1], None, ALU.mult
                            )
                            c1 = sp.tile([128, 512], F32, tag="cv1")
                            nc.vector.scalar_tensor_tensor(
                                c1[:], ue[:, 1:513], convw[j][:, 1:2], c0[:], ALU.mult, ALU.add
                            )
                            c2 = sp.tile([128, 512], F32, tag="cv0")
                            nc.vector.scalar_tensor_tensor(
                                c2[:], ue[:, 2:514], convw[j][:, 2:3], c1[:], ALU.mult, ALU.add
                            )
                            c3 = sp.tile([128, 512], F32, tag="cv1")
                            nc.vector.scalar_tensor_tensor(
                                c3[:], ue[:, 3:515], convw[j][:, 3:4], c2[:], ALU.mult, ALU.add
                            )
                            uc = sp.tile([128, 512], F32, tag="uc", bufs=2)
                            nc.scalar.activation(uc[:], c3[:], AF.Silu, bias=convb[j][:])
                            nc.sync.dma_start(
                                u_st[j * 128:(j + 1) * 128, tok0:tok0 + 512], uc[:]
                            )
                            nc.tensor.matmul(
                                px[:], wxpT[j][:], uc[:], start=(j == 0), stop=(j == NDT - 1)
                            )
                        nc.any.tensor_copy(x_dbl[:, tok0:tok0 + 512], px[:])

                # ================= phase B: AllReduce x_dbl =================
                nc.sync.dma_start(xdbl_in[:, :], x_dbl[:])
                if fake_cc:
                    nc.sync.dma_start(xdbl_out[:, :], xdbl_in[:, :])
                else:
                    nc.gpsimd.collective_compute(
                        "AllReduce",
                        ALU.add,
                        replica_groups=groups,
                        ins=[xdbl_in.opt()],
                        outs=[xdbl_out.opt()],
                    )
                xr_dt = sp.tile([DTR, T], F32, tag="xdbl")
                nc.sync.dma_start(xr_dt[:], xdbl_out[0:DTR, :])

                # ================= phases D/E: dt, scan, gate, out_proj =================
                with tc.tile_pool(name=f"psD{l}", bufs=2, space="PSUM") as psD:
                    for b in range(B):
                        bc0 = b * L
                        ysb = [None, None]
                        for j in range(NDT):
                            # softplus(xr) via y=e^x (x<=~0 here), series init,
                            # then 2 Newton steps for w=log(1+y):
                            #   w <- w + (1+y)e^-w - 1   (quadratic convergence)
                            yv = dp.tile([128, L], F32, tag="bt")
                            for q in range(4):
                                pdm = psD.tile([128, 2048], F32, tag="bc")
                                nc.tensor.matmul(
                                    pdm[:, 0:512],
                                    wdtT[:, j * 128:(j + 1) * 128],
                                    xr_dt[:, bc0 + q * 512: bc0 + (q + 1) * 512],
                                    start=True,
                                    stop=True,
                                )
                                nc.scalar.activation(
                                    yv[:, q * 512:(q + 1) * 512],
                                    pdm[:, 0:512],
                                    AF.Exp,
                                    bias=dtb[j][:],
                                )
                            y2 = dp.tile([128, L], F32, tag="hs")
                            nc.scalar.activation(y2[:], yv[:], AF.Square)
                            a1 = dp.tile([128, L], F32, tag="ada")
                            nc.vector.tensor_scalar(a1[:], yv[:], -0.5, 1.0, ALU.mult, ALU.add)
                            a2 = dp.tile([128, L], F32, tag="dtu")
                            nc.vector.tensor_mul(a2[:], yv[:], a1[:])
                            a3 = dp.tile([128, L], F32, tag="ada")
                            nc.vector.tensor_scalar(a3[:], yv[:], -0.25, 1.0 / 3.0, ALU.mult, ALU.add)
                            a4 = dp.tile([128, L], F32, tag="yacc", bufs=2)
                            nc.vector.tensor_mul(a4[:], y2[:], a3[:])
                            a5 = dp.tile([128, L], F32, tag="hs")
                            nc.vector.tensor_mul(a5[:], yv[:], a4[:])
                            w0r = dp.tile([128, L], F32, tag="yacc", bufs=2)
                            nc.vector.tensor_add(w0r[:], a2[:], a5[:])
                            w = dp.tile([128, L], F32, tag="hs")
                            nc.vector.tensor_scalar_max(w[:], w0r[:], 0.0)
                            dtt = None
                            for it, wtag in enumerate(["yacc", "hs", "dtt"]):
                                ew = dp.tile([128, L], F32, tag="ada")
                                nc.scalar.activation(ew[:], w[:], AF.Exp, scale=-1.0)
                                ye = dp.tile([128, L], F32, tag="yacc", bufs=2)
                                nc.vector.tensor_mul(ye[:], yv[:], ew[:])
                                tc_ = dp.tile([128, L], F32, tag="dtu")
                                nc.vector.scalar_tensor_tensor(
                                    tc_[:], ew[:], -1.0, ye[:], ALU.add, ALU.add
                                )
                                wn = dp.tile([128, L], F32, tag=wtag, bufs=2 if wtag == "yacc" else None)
                                nc.vector.tensor_add(wn[:], w[:], tc_[:])
                                w = wn
                            dtt = w
                            ub = dp.tile([128, L], F32, tag="ub")
                            nc.sync.dma_start(
                                ub[:], u_st[j * 128:(j + 1) * 128, bc0:bc0 + L]
                            )
                            dtu = dp.tile([128, L], F32, tag="dtu")
                            nc.vector.tensor_mul(dtu[:], dtt[:], ub[:])
                            yacc = None
                            for n in range(DS):
                                ada = dp.tile([128, L], F32, tag="ada")
                                nc.scalar.activation(
                                    ada[:], dtt[:], AF.Exp, scale=Asb[j][:, n:n + 1]
                                )
                                brow = dp.tile([1, L], F32, tag="bcrow", bufs=2)
                                nc.sync.dma_start(
                                    brow[:], xdbl_out[DTR + n:DTR + n + 1, bc0:bc0 + L]
                                )
                                pb = psD.tile([128, 2048], F32, tag="bc")
                                for q in range(4):
                                    nc.tensor.matmul(
                                        pb[:, q * 512:(q + 1) * 512],
                                        ones_sb[:],
                                        brow[:, q * 512:(q + 1) * 512],
                                        start=True,
                                        stop=True,
                                    )
                                bt = dp.tile([128, L], F32, tag="bt")
                                nc.vector.tensor_mul(bt[:], dtu[:], pb[:])
                                hs = dp.tile([128, L], F32, tag="hs")
                                nc.vector.tensor_tensor_scan(
                                    hs[:], ada[:], bt[:], 0.0, ALU.mult, ALU.add
                                )
                                crow = dp.tile([1, L], F32, tag="bcrow", bufs=2)
                                nc.sync.dma_start(
                                    crow[:],
                                    xdbl_out[DTR + DS + n:DTR + DS + n + 1, bc0:bc0 + L],
                                )
                                pc = psD.tile([128, 2048], F32, tag="bc")
                                for q in range(4):
                                    nc.tensor.matmul(
                                        pc[:, q * 512:(q + 1) * 512],
                                        ones_sb[:],
                                        crow[:, q * 512:(q + 1) * 512],
                                        start=True,
                                        stop=True,
                                    )
                                if n == 0:
                                    yacc = dp.tile([128, L], F32, tag="yacc", bufs=2)
                                    nc.vector.tensor_mul(yacc[:], hs[:], pc[:])
                                else:
                                    yt = dp.tile([128, L], F32, tag="ada")
                                    nc.vector.tensor_mul(yt[:], hs[:], pc[:])
                                    ynew = dp.tile([128, L], F32, tag="yacc", bufs=2)
                                    nc.vector.tensor_add(ynew[:], yacc[:], yt[:])
                                    yacc = ynew
                            zb = dp.tile([128, L], F32, tag="ada")
                            nc.sync.dma_start(
                                zb[:], z_st[j * 128:(j + 1) * 128, bc0:bc0 + L]
                            )
                            y1 = dp.tile([128, L], F32, tag="dtu")
                            nc.vector.scalar_tensor_tensor(
                                y1[:], ub[:], Dp[j][:], yacc[:], ALU.mult, ALU.add
                            )
                            y2 = dp.tile([128, L], F32, tag=f"ysb{j}")
                            nc.vector.tensor_mul(y2[:], y1[:], zb[:])
                            ysb[j] = y2
                        # out_proj for this batch
                        for tt in range(16):
                            for nt2 in range(2):
                                po = psD.tile([128, 512], F32, tag="bc")
                                for j in range(NDT):
                                    nc.tensor.matmul(
                                        po[:],
                                        ysb[j][:, tt * 128:(tt + 1) * 128],
                                        woutT[j][:, nt2 * 512:(nt2 + 1) * 512],
                                        start=(j == 0),
                                        stop=(j == NDT - 1),
                                    )
                                oc = dp.tile([128, 512], F32, tag="oc", bufs=2)
                                nc.any.tensor_copy(oc[:], po[:])
                                nc.sync.dma_start(
                                    out_part[bc0 + tt * 128: bc0 + (tt + 1) * 128,
                                             nt2 * 512:(nt2 + 1) * 512],
                                    oc[:],
                                )

                # ================= phase F: AllReduce layer output =================
                if fake_cc:
                    nc.sync.dma_start(hred[:, :], out_part[:, :])
                else:
                    nc.gpsimd.collective_compute(
                        "AllReduce",
                        ALU.add,
                        replica_groups=groups,
                        ins=[out_part.opt()],
                        outs=[hred.opt()],
                    )
                h_src = hred

            nc.sync.dma_start(out_dram[:, :], h_src[:, :])

    nc.compile()
    return nc


_CACHE = {}


def _get_nc(apply_norm_w, apply_norm_b, fake_cc=False):
    key = (apply_norm_w, apply_norm_b, fake_cc)
    if key not in _CACHE:
        _CACHE[key] = build_nc(apply_norm_w, apply_norm_b, fake_cc)
    return _CACHE[key]


def make_in_maps(x, norm_w, norm_b, in_proj_w, conv_w, conv_b, x_proj_w,
                 dt_proj_w, dt_proj_b, A_log, D, out_proj_w,
                 apply_norm_w, apply_norm_b):
    f = lambda a: np.ascontiguousarray(np.asarray(a), dtype=np.float32)
    x_tm = f(x).reshape(T, DM)
    in_maps = []
    for c in range(NCORES):
        sl = slice(c * DL, (c + 1) * DL)
        w_in_rows = np.concatenate(
            [np.asarray(in_proj_w)[:, sl, :], np.asarray(in_proj_w)[:, DI + c * DL: DI + (c + 1) * DL, :]],
            axis=1,
        )  # (2, 512, 1024)
        m = {
            "x_tm": x_tm,
            "w_inT": f(w_in_rows.transpose(0, 2, 1)),
            "w_outT": f(np.asarray(out_proj_w)[:, :, sl].transpose(0, 2, 1)),
            "w_xpT": f(np.asarray(x_proj_w)[:, :, sl].transpose(0, 2, 1)),
            "w_dtT": f(np.asarray(dt_proj_w)[:, sl, :].transpose(0, 2, 1)),
            "conv_w_c": f(np.asarray(conv_w)[:, sl, 0, :]),
            "conv_b_c": f(np.asarray(conv_b)[:, sl][..., None]),
            "dt_b_c": f(np.asarray(dt_proj_b)[:, sl][..., None]),
            "a_log_c": f(np.asarray(A_log)[:, sl, :]),
            "d_c": f(np.asarray(D)[:, sl][..., None]),
            "ident": np.eye(128, dtype=np.float32),
            "ones1": np.ones((1, 128), dtype=np.float32),
        }
        if apply_norm_w:
            m["norm_w_bc"] = f(np.broadcast_to(np.asarray(norm_w)[:, None, :], (DEPTH, 128, DM)))
        if apply_norm_b:
            m["norm_b_bc"] = f(np.broadcast_to(np.asarray(norm_b)[:, None, :], (DEPTH, 128, DM)))
        in_maps.append(m)
    return in_maps


def kernel(x, x_size, norm_w, norm_b, in_proj_w, conv_w, conv_b, x_proj_w,
           dt_proj_w, dt_proj_b, A_log, D, out_proj_w, **_unused):
    apply_norm_w = not np.allclose(np.asarray(norm_w), 1.0)
    apply_norm_b = not np.allclose(np.asarray(norm_b), 0.0)
    nc = _get_nc(apply_norm_w, apply_norm_b)
    in_maps = make_in_maps(
        x, norm_w, norm_b, in_proj_w, conv_w, conv_b, x_proj_w,
        dt_proj_w, dt_proj_b, A_log, D, out_proj_w,
        apply_norm_w, apply_norm_b,
    )
    res = run_bass_kernel_spmd(nc, in_maps, core_ids=list(range(NCORES)))
    return res.results[0]["out_tm"].reshape(B, L, DM)


# revision 21
# speedup vs baseline: 23.3809x; 23.3809x over previous
"""Trainium2 Bass kernel for a 2-layer Mamba stack (BasicLayer).

Per layer: LayerNorm -> in_proj (1024->4096) -> causal depthwise conv(k=4)
+ SiLU -> x_proj (2048->96) -> dt_proj + softplus -> selective scan over
L=2048 -> gate with SiLU(z) -> out_proj (2048->1024).

Sharding: tensor-parallel over d_inner (2048 / 8 cores = 256 channels per
core).  The selective scan is independent per channel, so each core scans
its own channels.  Cross-core sums (x_proj contraction and out_proj
contraction over d_inner) are AllReduced on-chip, split per batch so the
collectives overlap with compute.  Weights are pre-sliced and
pre-transposed on the host (pure data movement); all math runs on device.

The scan recurrence h_t = exp(dt*A)*h_{t-1} + (dt*u*B)_t runs on the DVE
tensor_tensor_scan instruction (fp32 internal state).  B_t / C_t rows are
replicated across partitions with broadcast DMAs.  softplus is computed
log-free via exp + series + Newton iterations (no Softplus/Ln table on
this hardware).
"""

import numpy as np

try:
    import concourse.bass as bass
except ImportError:  # pragma: no cover - fallback for odd sys.path setups
    import sys

    sys.path.insert(0, "/opt/trn_rl_repo")
    import concourse.bass as bass

import concourse.bacc as bacc
import concourse.mybir as mybir
import concourse.tile as tile
from concourse.bass_utils import run_bass_kernel_spmd

F32 = mybir.dt.float32
BF16 = mybir.dt.bfloat16
AF = mybir.ActivationFunctionType
ALU = mybir.AluOpType

# Problem shapes (hardcoded per the contract)
B, L = 2, 2048
DM, DI, DS, DTR, DCONV, DEPTH = 1024, 2048, 16, 64, 4, 2
EPS = 1e-5
NCORES = 8
DL = DI // NCORES          # 256 channels per core
NDT = DL // 128            # 2 channel tiles per core
T = B * L                  # 4096 tokens
NCH = T // 512             # 8 chunks of 512 tokens


def build_nc(apply_norm_w: bool, apply_norm_b: bool, fake_cc: bool = False,
             scan_bf16: bool = True):
    nc = bacc.Bacc(
        "TRN2",
        target_bir_lowering=False,
        debug=False,
        enable_asserts=False,
        num_devices=NCORES,
    )

    sdt = BF16 if scan_bf16 else F32

    # ---- I/O declarations (per-core data supplied via in_maps) ----
    x_dram = nc.dram_tensor("x_tm", [T, DM], F32, kind="ExternalInput")
    w_inT = nc.dram_tensor("w_inT", [DEPTH, DM, 4 * 128], F32, kind="ExternalInput")
    w_outT = nc.dram_tensor("w_outT", [DEPTH, DL, DM], F32, kind="ExternalInput")
    w_xpT = nc.dram_tensor("w_xpT", [DEPTH, DL, 96], F32, kind="ExternalInput")
    w_dtT = nc.dram_tensor("w_dtT", [DEPTH, DTR, DL], F32, kind="ExternalInput")
    conv_w = nc.dram_tensor("conv_w_c", [DEPTH, DL, DCONV], F32, kind="ExternalInput")
    conv_b = nc.dram_tensor("conv_b_c", [DEPTH, DL, 1], F32, kind="ExternalInput")
    dt_b = nc.dram_tensor("dt_b_c", [DEPTH, DL, 1], F32, kind="ExternalInput")
    a_log = nc.dram_tensor("a_log_c", [DEPTH, DL, DS], F32, kind="ExternalInput")
    d_p = nc.dram_tensor("d_c", [DEPTH, DL, 1], F32, kind="ExternalInput")
    ident = nc.dram_tensor("ident", [128, 128], F32, kind="ExternalInput")
    ones1 = nc.dram_tensor("ones1", [1, 128], F32, kind="ExternalInput")
    if apply_norm_w:
        nwb = nc.dram_tensor("norm_w_bc", [DEPTH, 128, DM], F32, kind="ExternalInput")
    if apply_norm_b:
        nbb = nc.dram_tensor("norm_b_bc", [DEPTH, 128, DM], F32, kind="ExternalInput")
    out_dram = nc.dram_tensor("out_tm", [T, DM], F32, kind="ExternalOutput")

    groups = [list(range(NCORES))]

    def all_reduce(src_ap, dst_ap):
        if fake_cc:
            nc.sync.dma_start(dst_ap, src_ap)
        else:
            nc.gpsimd.collective_compute(
                "AllReduce", ALU.add, replica_groups=groups,
                ins=[src_ap], outs=[dst_ap],
            )

    with tile.TileContext(nc, num_cores=NCORES) as tc:
        with (
            tc.tile_pool(name="wp", bufs=1) as wp,
            tc.tile_pool(name="lnp", bufs=1) as lnp,
            tc.tile_pool(name="sp", bufs=1) as sp,
            tc.tile_pool(name="dp", bufs=1) as dp,
            tc.tile_pool(name="dram", bufs=2, space="DRAM") as dram,
        ):
            ident_sb = wp.tile([128, 128], F32, tag="ident")
            nc.sync.dma_start(ident_sb[:], ident[:, :])
            idacc = ident_sb
            if scan_bf16:
                idbf = wp.tile([128, 128], BF16, tag="idbf")
                nc.vector.tensor_copy(idbf[:], ident_sb[:])
                idacc = idbf
            eps_sb = wp.tile([128, 1], F32, tag="eps")
            nc.vector.memset(eps_sb[:], EPS)

            hsrc = [x_dram.ap()[0:L, :], x_dram.ap()[L:T, :]]
            for l in range(DEPTH):
                # ---- per-layer weights ----
                winT = []
                for kt in range(8):
                    w = wp.tile([128, 512], F32, tag=f"winT{kt}")
                    nc.sync.dma_start(w[:], w_inT[l, kt * 128:(kt + 1) * 128, :])
                    winT.append(w)
                woutT = []
                for j in range(NDT):
                    w = wp.tile([128, DM], F32, tag=f"woutT{j}")
                    nc.sync.dma_start(w[:], w_outT[l, j * 128:(j + 1) * 128, :])
                    woutT.append(w)
                wxpT = []
                for j in range(NDT):
                    w = wp.tile([128, 96], F32, tag=f"wxpT{j}")
                    nc.sync.dma_start(w[:], w_xpT[l, j * 128:(j + 1) * 128, :])
                    wxpT.append(w)
                wdtT = wp.tile([DTR, DL], F32, tag="wdtT")
                nc.sync.dma_start(wdtT[:], w_dtT[l, :, :])
                convw, convb, dtb, Dp, Asb = [], [], [], [], []
                for j in range(NDT):
                    cw = wp.tile([128, DCONV], F32, tag=f"convw{j}")
                    nc.sync.dma_start(cw[:], conv_w[l, j * 128:(j + 1) * 128, :])
                    convw.append(cw)
                    cb = wp.tile([128, 1], F32, tag=f"convb{j}")
                    nc.sync.dma_start(cb[:], conv_b[l, j * 128:(j + 1) * 128, :])
                    convb.append(cb)
                    db = wp.tile([128, 1], F32, tag=f"dtb{j}")
                    nc.sync.dma_start(db[:], dt_b[l, j * 128:(j + 1) * 128, :])
                    dtb.append(db)
                    dd = wp.tile([128, 1], F32, tag=f"dd{j}")
                    nc.sync.dma_start(dd[:], d_p[l, j * 128:(j + 1) * 128, :])
                    Dp.append(dd)
                    at = wp.tile([128, DS], F32, tag=f"alog{j}")
                    nc.sync.dma_start(at[:], a_log[l, j * 128:(j + 1) * 128, :])
                    ae = wp.tile([128, DS], F32, tag=f"aexp{j}")
                    nc.scalar.activation(ae[:], at[:], AF.Exp)
                    an = wp.tile([128, DS], F32, tag=f"aneg{j}")
                    nc.vector.tensor_scalar_mul(an[:], ae[:], -1.0)
                    Asb.append(an)
                if apply_norm_w:
                    nw_sb = wp.tile([128, DM], F32, tag="nwsb")
                    nc.sync.dma_start(nw_sb[:], nwb[l, :, :])
                if apply_norm_b:
                    nb_sb = wp.tile([128, DM], F32, tag="nbsb")
                    nc.sync.dma_start(nb_sb[:], nbb[l, :, :])

                # ---- DRAM staging for this layer ----
                u_st = dram.tile([DL, T], F32, tag="ust")
                y_st = dram.tile([DL, T], F32, tag="yst")
                z_st = dram.tile([DL, T], F32, tag="zst")
                xdbl_in = [dram.tile([96, L], F32, tag=f"xdbli{b}", name=f"xdbli{l}_{b}") for b in range(B)]
                xdbl_sh = [dram.tile([96, L], F32, tag=f"xdblo{b}", addr_space="Shared",
                                      name=f"xdblo{l}_{b}") for b in range(B)]
                bc_bf = [dram.tile([2 * DS, L], sdt, tag=f"bcbf{b}", name=f"bcbf{l}_{b}") for b in range(B)]
                out_part = [dram.tile([L, DM], F32, tag=f"opart{b}", name=f"opart{l}_{b}") for b in range(B)]
                hred = [dram.tile([L, DM], F32, tag=f"hred{b}", addr_space="Shared",
                                   name=f"hred{l}_{b}") for b in range(B)]

                # ================= phase A: LN + transpose + in_proj + conv =================
                x_dbl = sp.tile([96, T], F32, tag="xdbl")
                prev_uext = [None, None]
                with tc.tile_pool(name=f"psA{l}", bufs=2, space="PSUM") as psA:
                    for ci in range(NCH):
                        b = ci // 4
                        tok0 = ci * 512
                        hn_pack = lnp.tile([128, 4096], F32, tag="hnpack")
                        for tti in range(4):
                            row0 = (ci % 4) * 512 + tti * 128
                            xa = lnp.tile([128, DM], F32, tag="xa", bufs=2)
                            nc.sync.dma_start(xa[:], hsrc[b][row0:row0 + 128, :])
                            st6 = lnp.tile([128, 12], F32, tag="st6", bufs=2)
                            nc.vector.bn_stats(st6[:, 0:6], xa[:, 0:512])
                            nc.vector.bn_stats(st6[:, 6:12], xa[:, 512:1024])
                            mv = lnp.tile([128, 2], F32, tag="mv", bufs=2)
                            nc.vector.bn_aggr(mv[:], st6[:].rearrange("p (g s) -> p g s", g=2))
                            std = lnp.tile([128, 1], F32, tag="std", bufs=2)
                            nc.scalar.activation(std[:], mv[:, 1:2], AF.Sqrt, bias=eps_sb[:])
                            rstd = lnp.tile([128, 1], F32, tag="rstd", bufs=2)
                            nc.vector.reciprocal(rstd[:], std[:])
                            nbias = lnp.tile([128, 1], F32, tag="nbias", bufs=2)
                            nc.vector.scalar_tensor_tensor(
                                nbias[:], mv[:, 0:1], -1.0, rstd[:], ALU.mult, ALU.mult
                            )
                            hcol = hn_pack[:, tti * DM:(tti + 1) * DM]
                            if apply_norm_w or apply_norm_b:
                                hn0 = lnp.tile([128, DM], F32, tag="hn0", bufs=2)
                                nc.scalar.activation(
                                    hn0[:], xa[:], AF.Identity, bias=nbias[:], scale=rstd[:]
                                )
                                if apply_norm_w and apply_norm_b:
                                    hn1 = lnp.tile([128, DM], F32, tag="hn1", bufs=2)
                                    nc.vector.tensor_mul(hn1[:], hn0[:], nw_sb[:])
                                    nc.vector.tensor_add(hcol, hn1[:], nb_sb[:])
                                elif apply_norm_w:
                                    nc.vector.tensor_mul(hcol, hn0[:], nw_sb[:])
                                else:
                                    nc.vector.tensor_add(hcol, hn0[:], nb_sb[:])
                            else:
                                nc.scalar.activation(
                                    hcol, xa[:], AF.Identity, bias=nbias[:], scale=rstd[:]
                                )
                        hnT = []
                        for kt in range(8):
                            pt = psA.tile([128, 512], F32, tag="pt")
                            for tti in range(4):
                                nc.tensor.transpose(
                                    pt[:, tti * 128:(tti + 1) * 128],
                                    hn_pack[:, tti * DM + kt * 128: tti * DM + (kt + 1) * 128],
                                    ident_sb[:],
                                )
                            ht = lnp.tile([128, 512], F32, tag=f"hnT{kt}")
                            nc.any.tensor_copy(ht[:], pt[:])
                            hnT.append(ht)
                        for mt in range(4):
                            pm = psA.tile([128, 512], F32, tag="pm")
                            for kt in range(8):
                                nc.tensor.matmul(
                                    pm[:],
                                    winT[kt][:, mt * 128:(mt + 1) * 128],
                                    hnT[kt][:],
                                    start=(kt == 0),
                                    stop=(kt == 7),
                                )
                            if mt < NDT:
                                ue = sp.tile([128, 515], F32, tag=f"uext{mt}", bufs=2)
                                if ci % 4 == 0:
                                    nc.vector.memset(ue[:, 0:3], 0.0)
                                else:
                                    nc.vector.tensor_copy(
                                        ue[:, 0:3], prev_uext[mt][:, 512:515]
                                    )
                                nc.any.tensor_copy(ue[:, 3:515], pm[:])
                                prev_uext[mt] = ue
                            else:
                                zc = sp.tile([128, 512], F32, tag="zc")
                                nc.scalar.activation(zc[:], pm[:], AF.Silu)
                                nc.sync.dma_start(
                                    z_st[(mt - NDT) * 128:(mt - NDT + 1) * 128, tok0:tok0 + 512],
                                    zc[:],
                                )
                        px = psA.tile([96, 512], F32, tag="px")
                        for j in range(NDT):
                            ue = prev_uext[j]
                            c0 = sp.tile([128, 512], F32, tag="cv0")
                            nc.vector.tensor_scalar(
                                c0[:], ue[:, 0:512], convw[j][:, 0:1], None, ALU.mult
                            )
                            c1 = sp.tile([128, 512], F32, tag="cv1")
                            nc.vector.scalar_tensor_tensor(
                                c1[:], ue[:, 1:513], convw[j][:, 1:2], c0[:], ALU.mult, ALU.add
                            )
                            c2 = sp.tile([128, 512], F32, tag="cv0")
                            nc.vector.scalar_tensor_tensor(
                                c2[:], ue[:, 2:514], convw[j][:, 2:3], c1[:], ALU.mult, ALU.add
                            )
                            c3 = sp.tile([128, 512], F32, tag="cv1")
                            nc.vector.scalar_tensor_tensor(
                                c3[:], ue[:, 3:515], convw[j][:, 3:4], c2[:], ALU.mult, ALU.add
                            )
                            uc = sp.tile([128, 512], F32, tag="uc", bufs=2)
                            nc.scalar.activation(uc[:], c3[:], AF.Silu, bias=convb[j][:])
                            nc.sync.dma_start(
                                u_st[j * 128:(j + 1) * 128, tok0:tok0 + 512], uc[:]
                            )
                            nc.tensor.matmul(
                                px[:], wxpT[j][:], uc[:], start=(j == 0), stop=(j == NDT - 1)
                            )
                        nc.any.tensor_copy(x_dbl[:, tok0:tok0 + 512], px[:])

                        # per-batch x_dbl AllReduce as soon as a batch's chunks finish
                        if ci % 4 == 3:
                            nc.sync.dma_start(xdbl_in[b][:, :], x_dbl[:, b * L:(b + 1) * L])
                            all_reduce(xdbl_in[b].opt(), xdbl_sh[b].opt())
                            # stage B/C rows (cast for the scan) back to DRAM for
                            # partition-broadcast loads
                            bcs = sp.tile([2 * DS, L], F32, tag="bcs")
                            nc.sync.dma_start(bcs[:], xdbl_sh[b][DTR:96, :])
                            bcsb = sp.tile([2 * DS, L], sdt, tag="bcsb")
                            nc.any.tensor_copy(bcsb[:], bcs[:])
                            nc.sync.dma_start(bc_bf[b][:, :], bcsb[:])

                # ================= phases D/E: dt, scan, gate, out_proj =================
                with tc.tile_pool(name=f"psD{l}", bufs=2, space="PSUM") as psD:
                    for b in range(B):
                        xrd = dp.tile([DTR, L], F32, tag="zb")
                        nc.sync.dma_start(xrd[:], xdbl_sh[b][0:DTR, :])
                        dtt, dtu = [], []
                        for j in range(NDT):
                            dt_j = dp.tile([128, L], F32, tag=f"dtt{j}")
                            # softplus(x) = log(1+e^x), log-free: y=e^x, series
                            # init, 3 Newton steps (w <- w + (1+y)e^-w - 1)
                            for hf in range(2):
                                h0 = hf * 1024
                                yv = dp.tile([128, 1024], F32, tag="sp0")
                                for q in range(2):
                                    pdm = psD.tile([128, 512], F32, tag="yps")
                                    nc.tensor.matmul(
                                        pdm[:],
                                        wdtT[:, j * 128:(j + 1) * 128],
                                        xrd[:, h0 + q * 512: h0 + (q + 1) * 512],
                                        start=True,
                                        stop=True,
                                    )
                                    nc.scalar.activation(
                                        yv[:, q * 512:(q + 1) * 512], pdm[:],
                                        AF.Exp, bias=dtb[j][:],
                                    )
                                y2s = dp.tile([128, 1024], F32, tag="sp1")
                                nc.scalar.activation(y2s[:], yv[:], AF.Square)
                                a1 = dp.tile([128, 1024], F32, tag="sp2")
                                nc.vector.tensor_scalar(a1[:], yv[:], -0.5, 1.0, ALU.mult, ALU.add)
                                a2 = dp.tile([128, 1024], F32, tag="sp3")
                                nc.vector.tensor_mul(a2[:], yv[:], a1[:])
                                a3 = dp.tile([128, 1024], F32, tag="sp2")
                                nc.vector.tensor_scalar(a3[:], yv[:], -0.25, 1.0 / 3.0, ALU.mult, ALU.add)
                                a4 = dp.tile([128, 1024], F32, tag="ada")
                                nc.vector.tensor_mul(a4[:], y2s[:], a3[:])
                                a5 = dp.tile([128, 1024], F32, tag="sp1")
                                nc.vector.tensor_mul(a5[:], yv[:], a4[:])
                                w0 = dp.tile([128, 1024], F32, tag="sp2")
                                nc.vector.tensor_add(w0[:], a2[:], a5[:])
                                w = dp.tile([128, 1024], F32, tag="sp3")
                                nc.vector.tensor_scalar_max(w[:], w0[:], 0.0)
                                for it, wtag in enumerate(["bt", "sp3", None]):
                                    ew = dp.tile([128, 1024], F32, tag="ada")
                                    nc.scalar.activation(ew[:], w[:], AF.Exp, scale=-1.0)
                                    ye = dp.tile([128, 1024], F32, tag="sp1")
                                    nc.vector.tensor_mul(ye[:], yv[:], ew[:])
                                    tcv = dp.tile([128, 1024], F32, tag="sp2")
                                    nc.vector.scalar_tensor_tensor(
                                        tcv[:], ew[:], -1.0, ye[:], ALU.add, ALU.add
                                    )
                                    if wtag is None:
                                        nc.vector.tensor_add(
                                            dt_j[:, h0:h0 + 1024], w[:], tcv[:]
                                        )
                                    else:
                                        wn = dp.tile([128, 1024], F32, tag=wtag)
                                        nc.vector.tensor_add(wn[:], w[:], tcv[:])
                                        w = wn
                            dtt.append(dt_j)
                            ub = dp.tile([128, L], F32, tag="ub")
                            nc.sync.dma_start(
                                ub[:], u_st[j * 128:(j + 1) * 128, b * L:(b + 1) * L]
                            )
                            du = dp.tile([128, L], sdt, tag=f"dtu{j}")
                            nc.vector.tensor_mul(du[:], dt_j[:], ub[:])
                            dtu.append(du)
                        y_ps = [psD.tile([128, L], F32, tag="yps", name=f"yps{l}_{b}_{jj}") for jj in range(NDT)]
                        for n in range(DS):
                            pb = dp.tile([128, L], sdt, tag="pbbf")
                            nc.sync.dma_start(
                                pb[:], bc_bf[b][n:n + 1, :].to_broadcast((128, L))
                            )
                            pc = dp.tile([128, L], sdt, tag="pcbf")
                            nc.sync.dma_start(
                                pc[:], bc_bf[b][DS + n:DS + n + 1, :].to_broadcast((128, L))
                            )
                            for j in range(NDT):
                                ada = dp.tile([128, L], F32, tag="ada")
                                nc.scalar.activation(
                                    ada[:], dtt[j][:], AF.Exp, scale=Asb[j][:, n:n + 1]
                                )
                                bt = dp.tile([128, L], sdt, tag="bt")
                                nc.vector.tensor_mul(bt[:], dtu[j][:], pb[:])
                                hs = dp.tile([128, L], sdt, tag="hs")
                                nc.vector.tensor_tensor_scan(
                                    hs[:], ada[:], bt[:], 0.0, ALU.mult, ALU.add
                                )
                                yt = dp.tile([128, L], sdt, tag="yt")
                                nc.vector.tensor_mul(yt[:], hs[:], pc[:])
                                for q in range(4):
                                    nc.tensor.matmul(
                                        y_ps[j][:, q * 512:(q + 1) * 512],
                                        idacc[:],
                                        yt[:, q * 512:(q + 1) * 512],
                                        start=(n == 0),
                                        stop=(n == DS - 1),
                                    )
                        for j in range(NDT):
                            ub2 = dp.tile([128, L], F32, tag="ub")
                            nc.sync.dma_start(
                                ub2[:], u_st[j * 128:(j + 1) * 128, b * L:(b + 1) * L]
                            )
                            zb = dp.tile([128, L], F32, tag="zb")
                            nc.sync.dma_start(
                                zb[:], z_st[j * 128:(j + 1) * 128, b * L:(b + 1) * L]
                            )
                            for hf in range(2):
                                h0 = hf * 1024
                                y1h = dp.tile([128, 1024], F32, tag="sp2")
                                nc.vector.scalar_tensor_tensor(
                                    y1h[:], ub2[:, h0:h0 + 1024], Dp[j][:],
                                    y_ps[j][:, h0:h0 + 1024], ALU.mult, ALU.add
                                )
                                y2h = dp.tile([128, 1024], F32, tag="sp3")
                                nc.vector.tensor_mul(y2h[:], y1h[:], zb[:, h0:h0 + 1024])
                                nc.sync.dma_start(
                                    y_st[j * 128:(j + 1) * 128,
                                         b * L + h0: b * L + h0 + 1024],
                                    y2h[:],
                                )
                        # out_proj for this batch
                        for tt in range(16):
                            yl = []
                            for j in range(NDT):
                                ylj = dp.tile([128, 128], F32, tag=f"yl{j}", bufs=2)
                                nc.sync.dma_start(
                                    ylj[:],
                                    y_st[j * 128:(j + 1) * 128,
                                         b * L + tt * 128: b * L + (tt + 1) * 128],
                                )
                                yl.append(ylj)
                            for nt2 in range(2):
                                po = psD.tile([128, 512], F32, tag="yps")
                                for j in range(NDT):
                                    nc.tensor.matmul(
                                        po[:],
                                        yl[j][:],
                                        woutT[j][:, nt2 * 512:(nt2 + 1) * 512],
                                        start=(j == 0),
                                        stop=(j == NDT - 1),
                                    )
                                oc = dp.tile([128, 512], F32, tag="oc")
                                nc.any.tensor_copy(oc[:], po[:])
                                nc.sync.dma_start(
                                    out_part[b][tt * 128:(tt + 1) * 128,
                                                nt2 * 512:(nt2 + 1) * 512],
                                    oc[:],
                                )
                        all_reduce(out_part[b].opt(), hred[b].opt())

                hsrc = [hred[0], hred[1]]

            for b in range(B):
                nc.sync.dma_start(out_dram[b * L:(b + 1) * L, :], hsrc[b])

    nc.compile()
    return nc


_CACHE = {}


def _get_nc(apply_norm_w, apply_norm_b, fake_cc=False, scan_bf16=True):
    key = (apply_norm_w, apply_norm_b, fake_cc, scan_bf16)
    if key not in _CACHE:
        _CACHE[key] = build_nc(apply_norm_w, apply_norm_b, fake_cc, scan_bf16)
    return _CACHE[key]


def make_in_maps(x, norm_w, norm_b, in_proj_w, conv_w, conv_b, x_proj_w,
                 dt_proj_w, dt_proj_b, A_log, D, out_proj_w,
                 apply_norm_w, apply_norm_b):
    f = lambda a: np.ascontiguousarray(np.asarray(a), dtype=np.float32)
    x_tm = f(x).reshape(T, DM)
    in_maps = []
    for c in range(NCORES):
        sl = slice(c * DL, (c + 1) * DL)
        w_in_rows = np.concatenate(
            [np.asarray(in_proj_w)[:, sl, :], np.asarray(in_proj_w)[:, DI + c * DL: DI + (c + 1) * DL, :]],
            axis=1,
        )  # (2, 512, 1024)
        m = {
            "x_tm": x_tm,
            "w_inT": f(w_in_rows.transpose(0, 2, 1)),
            "w_outT": f(np.asarray(out_proj_w)[:, :, sl].transpose(0, 2, 1)),
            "w_xpT": f(np.asarray(x_proj_w)[:, :, sl].transpose(0, 2, 1)),
            "w_dtT": f(np.asarray(dt_proj_w)[:, sl, :].transpose(0, 2, 1)),
            "conv_w_c": f(np.asarray(conv_w)[:, sl, 0, :]),
            "conv_b_c": f(np.asarray(conv_b)[:, sl][..., None]),
            "dt_b_c": f(np.asarray(dt_proj_b)[:, sl][..., None]),
            "a_log_c": f(np.asarray(A_log)[:, sl, :]),
            "d_c": f(np.asarray(D)[:, sl][..., None]),
            "ident": np.eye(128, dtype=np.float32),
            "ones1": np.ones((1, 128), dtype=np.float32),
        }
        if apply_norm_w:
            m["norm_w_bc"] = f(np.broadcast_to(np.asarray(norm_w)[:, None, :], (DEPTH, 128, DM)))
        if apply_norm_b:
            m["norm_b_bc"] = f(np.broadcast_to(np.asarray(norm_b)[:, None, :], (DEPTH, 128, DM)))
        in_maps.append(m)
    return in_maps


def kernel(x, x_size, norm_w, norm_b, in_proj_w, conv_w, conv_b, x_proj_w,
           dt_proj_w, dt_proj_b, A_log, D, out_proj_w, **_unused):
    apply_norm_w = not np.allclose(np.asarray(norm_w), 1.0)
    apply_norm_b = not np.allclose(np.asarray(norm_b), 0.0)
    nc = _get_nc(apply_norm_w, apply_norm_b)
    in_maps = make_in_maps(
        x, norm_w, norm_b, in_proj_w, conv_w, conv_b, x_proj_w,
        dt_proj_w, dt_proj_b, A_log, D, out_proj_w,
        apply_norm_w, apply_norm_b,
    )
    res = run_bass_kernel_spmd(nc, in_maps, core_ids=list(range(NCORES)))
    return res.results[0]["out_tm"].reshape(B, L, DM)


# revision 26
# speedup vs baseline: 27.6032x; 1.1806x over previous
"""Trainium2 Bass kernel for a 2-layer Mamba stack (BasicLayer).

Per layer: LayerNorm -> in_proj (1024->4096) -> causal depthwise conv(k=4)
+ SiLU -> x_proj (2048->96) -> dt_proj + softplus -> selective scan over
L=2048 -> gate with SiLU(z) -> out_proj (2048->1024).

Sharding: tensor-parallel over d_inner (2048 / 8 cores = 256 channels per
core).  The selective scan is independent per channel, so each core scans
its own channels.  Cross-core sums (x_proj contraction and out_proj
contraction over d_inner) are AllReduced on-chip, split per batch so the
collectives overlap with compute.  Weights are pre-sliced and
pre-transposed on the host (pure data movement); all math runs on device.

The scan recurrence h_t = exp(dt*A)*h_{t-1} + (dt*u*B)_t runs on the DVE
tensor_tensor_scan instruction (fp32 internal state).  B_t / C_t rows are
replicated across partitions with broadcast DMAs.  softplus is computed
log-free via exp + series + Newton iterations (no Softplus/Ln table on
this hardware).
"""

import numpy as np

try:
    import concourse.bass as bass
except ImportError:  # pragma: no cover - fallback for odd sys.path setups
    import sys

    sys.path.insert(0, "/opt/trn_rl_repo")
    import concourse.bass as bass

import concourse.bacc as bacc
import concourse.mybir as mybir
import concourse.tile as tile
from concourse.bass_utils import run_bass_kernel_spmd

F32 = mybir.dt.float32
BF16 = mybir.dt.bfloat16
AF = mybir.ActivationFunctionType
ALU = mybir.AluOpType

# Problem shapes (hardcoded per the contract)
B, L = 2, 2048
DM, DI, DS, DTR, DCONV, DEPTH = 1024, 2048, 16, 64, 4, 2
EPS = 1e-5
NCORES = 8
DL = DI // NCORES          # 256 channels per core
NDT = DL // 128            # 2 channel tiles per core
T = B * L                  # 4096 tokens
NCH = T // 512             # 8 chunks of 512 tokens


def build_nc(apply_norm_w: bool, apply_norm_b: bool, fake_cc: bool = False,
             scan_bf16: bool = True):
    nc = bacc.Bacc(
        "TRN2",
        target_bir_lowering=False,
        debug=False,
        enable_asserts=False,
        num_devices=NCORES,
    )

    sdt = BF16 if scan_bf16 else F32

    # ---- I/O declarations (per-core data supplied via in_maps) ----
    x_dram = nc.dram_tensor("x_tm", [T, DM], F32, kind="ExternalInput")
    w_inT = nc.dram_tensor("w_inT", [DEPTH, DM, 4 * 128], F32, kind="ExternalInput")
    w_outT = nc.dram_tensor("w_outT", [DEPTH, DL, DM], F32, kind="ExternalInput")
    w_xpT = nc.dram_tensor("w_xpT", [DEPTH, DL, 96], F32, kind="ExternalInput")
    w_dtT = nc.dram_tensor("w_dtT", [DEPTH, DTR, DL], F32, kind="ExternalInput")
    conv_w = nc.dram_tensor("conv_w_c", [DEPTH, DL, DCONV], F32, kind="ExternalInput")
    conv_b = nc.dram_tensor("conv_b_c", [DEPTH, DL, 1], F32, kind="ExternalInput")
    dt_b = nc.dram_tensor("dt_b_c", [DEPTH, DL, 1], F32, kind="ExternalInput")
    a_log = nc.dram_tensor("a_log_c", [DEPTH, DL, DS], F32, kind="ExternalInput")
    d_p = nc.dram_tensor("d_c", [DEPTH, DL, 1], F32, kind="ExternalInput")
    ident = nc.dram_tensor("ident", [128, 128], F32, kind="ExternalInput")
    ones1 = nc.dram_tensor("ones1", [1, 128], F32, kind="ExternalInput")
    if apply_norm_w:
        nwb = nc.dram_tensor("norm_w_bc", [DEPTH, 128, DM], F32, kind="ExternalInput")
    if apply_norm_b:
        nbb = nc.dram_tensor("norm_b_bc", [DEPTH, 128, DM], F32, kind="ExternalInput")
    out_dram = nc.dram_tensor("out_tm", [T, DM], F32, kind="ExternalOutput")

    groups = [list(range(NCORES))]

    def all_reduce(src_ap, dst_ap):
        if fake_cc:
            nc.sync.dma_start(dst_ap, src_ap)
        else:
            nc.gpsimd.collective_compute(
                "AllReduce", ALU.add, replica_groups=groups,
                ins=[src_ap], outs=[dst_ap],
            )

    with tile.TileContext(nc, num_cores=NCORES) as tc:
        with (
            tc.tile_pool(name="wp", bufs=1) as wp,
            tc.tile_pool(name="lnp", bufs=1) as lnp,
            tc.tile_pool(name="sp", bufs=1) as sp,
            tc.tile_pool(name="dp", bufs=1) as dp,
            tc.tile_pool(name="dram", bufs=2, space="DRAM") as dram,
        ):
            ident_sb = wp.tile([128, 128], F32, tag="ident")
            nc.sync.dma_start(ident_sb[:], ident[:, :])
            idacc = ident_sb
            if scan_bf16:
                idbf = wp.tile([128, 128], BF16, tag="idbf")
                nc.vector.tensor_copy(idbf[:], ident_sb[:])
                idacc = idbf
            eps_sb = wp.tile([128, 1], F32, tag="eps")
            nc.vector.memset(eps_sb[:], EPS)

            hsrc = [x_dram.ap()[0:L, :], x_dram.ap()[L:T, :]]
            psA_cm = tc.tile_pool(name="psA", bufs=2, space="PSUM")
            psA = psA_cm.__enter__()
            psD_cm = tc.tile_pool(name="psD", bufs=1, space="PSUM")
            psD = psD_cm.__enter__()
            for l in range(DEPTH):
                # ---- per-layer weights ----
                winT = []
                for kt in range(8):
                    w = wp.tile([128, 512], F32, tag=f"winT{kt}")
                    nc.sync.dma_start(w[:], w_inT[l, kt * 128:(kt + 1) * 128, :])
                    winT.append(w)
                woutT = []
                for j in range(NDT):
                    w = wp.tile([128, DM], F32, tag=f"woutT{j}")
                    nc.sync.dma_start(w[:], w_outT[l, j * 128:(j + 1) * 128, :])
                    woutT.append(w)
                wxpT = []
                for j in range(NDT):
                    w = wp.tile([128, 96], F32, tag=f"wxpT{j}")
                    nc.sync.dma_start(w[:], w_xpT[l, j * 128:(j + 1) * 128, :])
                    wxpT.append(w)
                wdtT = wp.tile([DTR, DL], F32, tag="wdtT")
                nc.sync.dma_start(wdtT[:], w_dtT[l, :, :])
                convw, convb, dtb, Dp, Asb = [], [], [], [], []
                for j in range(NDT):
                    cw = wp.tile([128, DCONV], F32, tag=f"convw{j}")
                    nc.sync.dma_start(cw[:], conv_w[l, j * 128:(j + 1) * 128, :])
                    convw.append(cw)
                    cb = wp.tile([128, 1], F32, tag=f"convb{j}")
                    nc.sync.dma_start(cb[:], conv_b[l, j * 128:(j + 1) * 128, :])
                    convb.append(cb)
                    db = wp.tile([128, 1], F32, tag=f"dtb{j}")
                    nc.sync.dma_start(db[:], dt_b[l, j * 128:(j + 1) * 128, :])
                    dtb.append(db)
                    dd = wp.tile([128, 1], F32, tag=f"dd{j}")
                    nc.sync.dma_start(dd[:], d_p[l, j * 128:(j + 1) * 128, :])
                    Dp.append(dd)
                    at = wp.tile([128, DS], F32, tag=f"alog{j}")
                    nc.sync.dma_start(at[:], a_log[l, j * 128:(j + 1) * 128, :])
                    ae = wp.tile([128, DS], F32, tag=f"aexp{j}")
                    nc.scalar.activation(ae[:], at[:], AF.Exp)
                    an = wp.tile([128, DS], F32, tag=f"aneg{j}")
                    nc.vector.tensor_scalar_mul(an[:], ae[:], -1.0)
                    Asb.append(an)
                if apply_norm_w:
                    nw_sb = wp.tile([128, DM], F32, tag="nwsb")
                    nc.sync.dma_start(nw_sb[:], nwb[l, :, :])
                if apply_norm_b:
                    nb_sb = wp.tile([128, DM], F32, tag="nbsb")
                    nc.sync.dma_start(nb_sb[:], nbb[l, :, :])

                # ---- DRAM staging for this layer ----
                u_st = dram.tile([DL, T], F32, tag="ust")
                y_st = dram.tile([DL, T], F32, tag="yst")
                z_st = dram.tile([DL, T], F32, tag="zst")
                xdbl_in = [dram.tile([96, L], F32, tag=f"xdbli{b}", name=f"xdbli{l}_{b}") for b in range(B)]
                xdbl_sh = [dram.tile([96, L], F32, tag=f"xdblo{b}", addr_space="Shared",
                                      name=f"xdblo{l}_{b}") for b in range(B)]
                bc_bf = [dram.tile([2 * DS, L], sdt, tag=f"bcbf{b}", name=f"bcbf{l}_{b}") for b in range(B)]
                out_part = [dram.tile([L, DM], F32, tag=f"opart{b}", name=f"opart{l}_{b}") for b in range(B)]
                hred = [dram.tile([L, DM], F32, tag=f"hred{b}", addr_space="Shared",
                                   name=f"hred{l}_{b}") for b in range(B)]

                # ================= phase A: LN + transpose + in_proj + conv =================
                x_dbl = sp.tile([96, T], F32, tag="xdbl")
                prev_uext = [None, None]
                if True:
                    for ci in range(NCH):
                        b = ci // 4
                        tok0 = ci * 512
                        hn_pack = lnp.tile([128, 4096], F32, tag="hnpack")
                        for tti in range(4):
                            row0 = (ci % 4) * 512 + tti * 128
                            xa = lnp.tile([128, DM], F32, tag="xa", bufs=2)
                            nc.sync.dma_start(xa[:], hsrc[b][row0:row0 + 128, :])
                            hcol = hn_pack[:, tti * DM:(tti + 1) * DM]
                            sums = lnp.tile([128, 1], F32, tag="sums", bufs=2)
                            nc.scalar.activation(hcol, xa[:], AF.Identity, accum_out=sums[:])
                            sumsq = lnp.tile([128, 1], F32, tag="sumsq", bufs=2)
                            nc.scalar.activation(hcol, xa[:], AF.Square, accum_out=sumsq[:])
                            mean = lnp.tile([128, 1], F32, tag="mean", bufs=2)
                            nc.vector.tensor_scalar_mul(mean[:], sums[:], 1.0 / DM)
                            msq = lnp.tile([128, 1], F32, tag="msq", bufs=2)
                            nc.vector.tensor_scalar_mul(msq[:], sumsq[:], 1.0 / DM)
                            nvar = lnp.tile([128, 1], F32, tag="nvar", bufs=2)
                            nc.vector.scalar_tensor_tensor(
                                nvar[:], mean[:], mean[:], msq[:], ALU.mult, ALU.subtract
                            )
                            std = lnp.tile([128, 1], F32, tag="std", bufs=2)
                            nc.scalar.activation(std[:], nvar[:], AF.Sqrt, bias=eps_sb[:], scale=-1.0)
                            rstd = lnp.tile([128, 1], F32, tag="rstd", bufs=2)
                            nc.vector.reciprocal(rstd[:], std[:])
                            nbias = lnp.tile([128, 1], F32, tag="nbias", bufs=2)
                            nc.vector.scalar_tensor_tensor(
                                nbias[:], mean[:], -1.0, rstd[:], ALU.mult, ALU.mult
                            )
                            if apply_norm_w or apply_norm_b:
                                hn0 = lnp.tile([128, DM], F32, tag="hn0", bufs=2)
                                nc.scalar.activation(
                                    hn0[:], xa[:], AF.Identity, bias=nbias[:], scale=rstd[:]
                                )
                                if apply_norm_w and apply_norm_b:
                                    hn1 = lnp.tile([128, DM], F32, tag="hn1", bufs=2)
                                    nc.vector.tensor_mul(hn1[:], hn0[:], nw_sb[:])
                                    nc.vector.tensor_add(hcol, hn1[:], nb_sb[:])
                                elif apply_norm_w:
                                    nc.vector.tensor_mul(hcol, hn0[:], nw_sb[:])
                                else:
                                    nc.vector.tensor_add(hcol, hn0[:], nb_sb[:])
                            else:
                                nc.scalar.activation(
                                    hcol, xa[:], AF.Identity, bias=nbias[:], scale=rstd[:]
                                )
                        hnT = []
                        for kt in range(8):
                            pt = psA.tile([128, 512], F32, tag="pt", bufs=1)
                            for tti in range(4):
                                nc.tensor.transpose(
                                    pt[:, tti * 128:(tti + 1) * 128],
                                    hn_pack[:, tti * DM + kt * 128: tti * DM + (kt + 1) * 128],
                                    ident_sb[:],
                                )
                            ht = lnp.tile([128, 512], F32, tag=f"hnT{kt}")
                            nc.any.tensor_copy(ht[:], pt[:])
                            hnT.append(ht)
                        for mt in range(4):
                            pm = psA.tile([128, 512], F32, tag="pm")
                            for kt in range(8):
                                nc.tensor.matmul(
                                    pm[:],
                                    winT[kt][:, mt * 128:(mt + 1) * 128],
                                    hnT[kt][:],
                                    start=(kt == 0),
                                    stop=(kt == 7),
                                )
                            if mt < NDT:
                                ue = sp.tile([128, 515], F32, tag=f"uext{mt}", bufs=2)
                                if ci % 4 == 0:
                                    nc.vector.memset(ue[:, 0:3], 0.0)
                                else:
                                    nc.vector.tensor_copy(
                                        ue[:, 0:3], prev_uext[mt][:, 512:515]
                                    )
                                nc.any.tensor_copy(ue[:, 3:515], pm[:])
                                prev_uext[mt] = ue
                            else:
                                zc = sp.tile([128, 512], F32, tag="zc")
                                nc.scalar.activation(zc[:], pm[:], AF.Silu)
                                nc.sync.dma_start(
                                    z_st[(mt - NDT) * 128:(mt - NDT + 1) * 128, tok0:tok0 + 512],
                                    zc[:],
                                )
                        px = psA.tile([96, 512], F32, tag="pm")
                        for j in range(NDT):
                            ue = prev_uext[j]
                            c0 = sp.tile([128, 512], F32, tag="cv0")
                            nc.vector.tensor_scalar(
                                c0[:], ue[:, 0:512], convw[j][:, 0:1], None, ALU.mult
                            )
                            c1 = sp.tile([128, 512], F32, tag="cv1")
                            nc.vector.scalar_tensor_tensor(
                                c1[:], ue[:, 1:513], convw[j][:, 1:2], c0[:], ALU.mult, ALU.add
                            )
                            c2 = sp.tile([128, 512], F32, tag="cv0")
                            nc.vector.scalar_tensor_tensor(
                                c2[:], ue[:, 2:514], convw[j][:, 2:3], c1[:], ALU.mult, ALU.add
                            )
                            c3 = sp.tile([128, 512], F32, tag="cv1")
                            nc.vector.scalar_tensor_tensor(
                                c3[:], ue[:, 3:515], convw[j][:, 3:4], c2[:], ALU.mult, ALU.add
                            )
                            uc = sp.tile([128, 512], F32, tag="uc", bufs=2)
                            nc.scalar.activation(uc[:], c3[:], AF.Silu, bias=convb[j][:])
                            nc.sync.dma_start(
                                u_st[j * 128:(j + 1) * 128, tok0:tok0 + 512], uc[:]
                            )
                            nc.tensor.matmul(
                                px[:], wxpT[j][:], uc[:], start=(j == 0), stop=(j == NDT - 1)
                            )
                        nc.any.tensor_copy(x_dbl[:, tok0:tok0 + 512], px[:])

                        # per-batch x_dbl AllReduce as soon as a batch's chunks finish
                        if ci % 4 == 3:
                            nc.sync.dma_start(xdbl_in[b][:, :], x_dbl[:, b * L:(b + 1) * L])
                            all_reduce(xdbl_in[b].opt(), xdbl_sh[b].opt())
                            # stage B/C rows (cast for the scan) back to DRAM for
                            # partition-broadcast loads
                            bcs = sp.tile([2 * DS, L], F32, tag="bcs")
                            nc.sync.dma_start(bcs[:], xdbl_sh[b][DTR:96, :])
                            bcsb = sp.tile([2 * DS, L], sdt, tag="bcsb")
                            nc.any.tensor_copy(bcsb[:], bcs[:])
                            nc.sync.dma_start(bc_bf[b][:, :], bcsb[:])

                    # ============= phases D/E: dt, scan, gate, out_proj =============
                    for b in range(B):
                        xrd = dp.tile([DTR, L], F32, tag="xrd")
                        nc.sync.dma_start(xrd[:], xdbl_sh[b][0:DTR, :])
                        for j in range(NDT):
                            dt_j = dp.tile([128, L], F32, tag="dtt")
                            # softplus(x) = log(1+e^x), log-free: y=e^x, series
                            # init, 3 Newton steps (w <- w + (1+y)e^-w - 1)
                            for hf in range(2):
                                h0 = hf * 1024
                                yv = dp.tile([128, 1024], F32, tag="sp0")
                                for q in range(2):
                                    pdm = psD.tile([128, 512], F32, tag="yps")
                                    nc.tensor.matmul(
                                        pdm[:],
                                        wdtT[:, j * 128:(j + 1) * 128],
                                        xrd[:, h0 + q * 512: h0 + (q + 1) * 512],
                                        start=True,
                                        stop=True,
                                    )
                                    nc.scalar.activation(
                                        yv[:, q * 512:(q + 1) * 512], pdm[:],
                                        AF.Exp, bias=dtb[j][:],
                                    )
                                y2s = dp.tile([128, 1024], F32, tag="sp1")
                                nc.scalar.activation(y2s[:], yv[:], AF.Square)
                                a1 = dp.tile([128, 1024], F32, tag="sp2")
                                nc.vector.tensor_scalar(a1[:], yv[:], -0.5, 1.0, ALU.mult, ALU.add)
                                a2 = dp.tile([128, 1024], F32, tag="sp3")
                                nc.vector.tensor_mul(a2[:], yv[:], a1[:])
                                a3 = dp.tile([128, 1024], F32, tag="sp2")
                                nc.vector.tensor_scalar(a3[:], yv[:], -0.25, 1.0 / 3.0, ALU.mult, ALU.add)
                                a4 = dp.tile([128, 1024], F32, tag="ada")
                                nc.vector.tensor_mul(a4[:], y2s[:], a3[:])
                                a5 = dp.tile([128, 1024], F32, tag="sp1")
                                nc.vector.tensor_mul(a5[:], yv[:], a4[:])
                                w0 = dp.tile([128, 1024], F32, tag="sp2")
                                nc.vector.tensor_add(w0[:], a2[:], a5[:])
                                w = dp.tile([128, 1024], F32, tag="sp3")
                                nc.vector.tensor_scalar_max(w[:], w0[:], 0.0)
                                for it, wtag in enumerate(["bt", None]):
                                    ew = dp.tile([128, 1024], F32, tag="ada")
                                    nc.scalar.activation(ew[:], w[:], AF.Exp, scale=-1.0)
                                    ye = dp.tile([128, 1024], F32, tag="sp1")
                                    nc.vector.tensor_mul(ye[:], yv[:], ew[:])
                                    tcv = dp.tile([128, 1024], F32, tag="sp2")
                                    nc.vector.scalar_tensor_tensor(
                                        tcv[:], ew[:], -1.0, ye[:], ALU.add, ALU.add
                                    )
                                    if wtag is None:
                                        nc.vector.tensor_add(
                                            dt_j[:, h0:h0 + 1024], w[:], tcv[:]
                                        )
                                    else:
                                        wn = dp.tile([128, 1024], F32, tag=wtag)
                                        nc.vector.tensor_add(wn[:], w[:], tcv[:])
                                        w = wn
                            ub = dp.tile([128, L], F32, tag="ub")
                            nc.sync.dma_start(
                                ub[:], u_st[j * 128:(j + 1) * 128, b * L:(b + 1) * L]
                            )
                            du = dp.tile([128, L], sdt, tag="dtu")
                            nc.vector.tensor_mul(du[:], dt_j[:], ub[:])
                            y_ps = psD.tile([128, L], F32, tag="yps")
                            for n in range(DS):
                                pb = dp.tile([128, L], sdt, tag="pbbf", bufs=2)
                                nc.sync.dma_start(
                                    pb[:], bc_bf[b][n:n + 1, :].to_broadcast((128, L))
                                )
                                pc = dp.tile([128, L], sdt, tag="pcbf", bufs=2)
                                nc.sync.dma_start(
                                    pc[:], bc_bf[b][DS + n:DS + n + 1, :].to_broadcast((128, L))
                                )
                                ada = dp.tile([128, L], sdt, tag="adas")
                                nc.scalar.activation(
                                    ada[:], dt_j[:], AF.Exp, scale=Asb[j][:, n:n + 1]
                                )
                                bt = dp.tile([128, L], sdt, tag="bt")
                                nc.vector.tensor_mul(bt[:], du[:], pb[:])
                                hs = dp.tile([128, L], sdt, tag="hs")
                                nc.vector.tensor_tensor_scan(
                                    hs[:], ada[:], bt[:], 0.0, ALU.mult, ALU.add
                                )
                                yt = dp.tile([128, L], sdt, tag="yt")
                                nc.vector.tensor_mul(yt[:], hs[:], pc[:])
                                for q in range(4):
                                    nc.tensor.matmul(
                                        y_ps[:, q * 512:(q + 1) * 512],
                                        idacc[:],
                                        yt[:, q * 512:(q + 1) * 512],
                                        start=(n == 0),
                                        stop=(n == DS - 1),
                                    )
                            ub2 = dp.tile([128, L], F32, tag="ub")
                            nc.sync.dma_start(
                                ub2[:], u_st[j * 128:(j + 1) * 128, b * L:(b + 1) * L]
                            )
                            zb = dp.tile([128, L], F32, tag="zb")
                            nc.sync.dma_start(
                                zb[:], z_st[j * 128:(j + 1) * 128, b * L:(b + 1) * L]
                            )
                            for hf in range(2):
                                h0 = hf * 1024
                                y1h = dp.tile([128, 1024], F32, tag="sp2")
                                nc.vector.scalar_tensor_tensor(
                                    y1h[:], ub2[:, h0:h0 + 1024], Dp[j][:],
                                    y_ps[:, h0:h0 + 1024], ALU.mult, ALU.add
                                )
                                y2h = dp.tile([128, 1024], F32, tag="sp3")
                                nc.vector.tensor_mul(y2h[:], y1h[:], zb[:, h0:h0 + 1024])
                                nc.sync.dma_start(
                                    y_st[j * 128:(j + 1) * 128,
                                         b * L + h0: b * L + h0 + 1024],
                                    y2h[:],
                                )
                        # out_proj for this batch
                        for tt in range(16):
                            yl = []
                            for j in range(NDT):
                                ylj = dp.tile([128, 128], F32, tag=f"yl{j}", bufs=2)
                                nc.sync.dma_start(
                                    ylj[:],
                                    y_st[j * 128:(j + 1) * 128,
                                         b * L + tt * 128: b * L + (tt + 1) * 128],
                                )
                                yl.append(ylj)
                            for nt2 in range(2):
                                po = psD.tile([128, 512], F32, tag="po")
                                for j in range(NDT):
                                    nc.tensor.matmul(
                                        po[:],
                                        yl[j][:],
                                        woutT[j][:, nt2 * 512:(nt2 + 1) * 512],
                                        start=(j == 0),
                                        stop=(j == NDT - 1),
                                    )
                                oc = dp.tile([128, 512], F32, tag="oc")
                                nc.any.tensor_copy(oc[:], po[:])
                                nc.sync.dma_start(
                                    out_part[b][tt * 128:(tt + 1) * 128,
                                                nt2 * 512:(nt2 + 1) * 512],
                                    oc[:],
                                )
                        all_reduce(out_part[b].opt(), hred[b].opt())

                hsrc = [hred[0], hred[1]]

            for b in range(B):
                nc.sync.dma_start(out_dram[b * L:(b + 1) * L, :], hsrc[b])
            psD_cm.__exit__(None, None, None)
            psA_cm.__exit__(None, None, None)

    nc.compile()
    return nc


_CACHE = {}


def _get_nc(apply_norm_w, apply_norm_b, fake_cc=False, scan_bf16=True):
    key = (apply_norm_w, apply_norm_b, fake_cc, scan_bf16)
    if key not in _CACHE:
        _CACHE[key] = build_nc(apply_norm_w, apply_norm_b, fake_cc, scan_bf16)
    return _CACHE[key]


def make_in_maps(x, norm_w, norm_b, in_proj_w, conv_w, conv_b, x_proj_w,
                 dt_proj_w, dt_proj_b, A_log, D, out_proj_w,
                 apply_norm_w, apply_norm_b):
    f = lambda a: np.ascontiguousarray(np.asarray(a), dtype=np.float32)
    x_tm = f(x).reshape(T, DM)
    in_maps = []
    for c in range(NCORES):
        sl = slice(c * DL, (c + 1) * DL)
        w_in_rows = np.concatenate(
            [np.asarray(in_proj_w)[:, sl, :], np.asarray(in_proj_w)[:, DI + c * DL: DI + (c + 1) * DL, :]],
            axis=1,
        )  # (2, 512, 1024)
        m = {
            "x_tm": x_tm,
            "w_inT": f(w_in_rows.transpose(0, 2, 1)),
            "w_outT": f(np.asarray(out_proj_w)[:, :, sl].transpose(0, 2, 1)),
            "w_xpT": f(np.asarray(x_proj_w)[:, :, sl].transpose(0, 2, 1)),
            "w_dtT": f(np.asarray(dt_proj_w)[:, sl, :].transpose(0, 2, 1)),
            "conv_w_c": f(np.asarray(conv_w)[:, sl, 0, :]),
            "conv_b_c": f(np.asarray(conv_b)[:, sl][..., None]),
            "dt_b_c": f(np.asarray(dt_proj_b)[:, sl][..., None]),
            "a_log_c": f(np.asarray(A_log)[:, sl, :]),
            "d_c": f(np.asarray(D)[:, sl][..., None]),
            "ident": np.eye(128, dtype=np.float32),
            "ones1": np.ones((1, 128), dtype=np.float32),
        }
        if apply_norm_w:
            m["norm_w_bc"] = f(np.broadcast_to(np.asarray(norm_w)[:, None, :], (DEPTH, 128, DM)))
        if apply_norm_b:
            m["norm_b_bc"] = f(np.broadcast_to(np.asarray(norm_b)[:, None, :], (DEPTH, 128, DM)))
        in_maps.append(m)
    return in_maps


def kernel(x, x_size, norm_w, norm_b, in_proj_w, conv_w, conv_b, x_proj_w,
           dt_proj_w, dt_proj_b, A_log, D, out_proj_w, **_unused):
    apply_norm_w = not np.allclose(np.asarray(norm_w), 1.0)
    apply_norm_b = not np.allclose(np.asarray(norm_b), 0.0)
    nc = _get_nc(apply_norm_w, apply_norm_b)
    in_maps = make_in_maps(
        x, norm_w, norm_b, in_proj_w, conv_w, conv_b, x_proj_w,
        dt_proj_w, dt_proj_b, A_log, D, out_proj_w,
        apply_norm_w, apply_norm_b,
    )
    res = run_bass_kernel_spmd(nc, in_maps, core_ids=list(range(NCORES)))
    return res.results[0]["out_tm"].reshape(B, L, DM)
